# revision 1
# baseline (speedup 1.0000x reference)
"""Trainium2 Bass kernel for the batched quantum-gate problem.

Math: the reference computes, per batch element b,
    out_b = expm(-i*dt_b*H_rot) @ expm(-i*zeta*H_kick) @ s_b
with H_kick = kron(a + a^T, sigma_x), H_rot = kron(a @ a^T, I2), dt = X + time.

H_rot = kron(M, I2) with M = a @ a^T symmetric PSD.  With M = U diag(lam) U^T:
    expm(-i*t*H_rot) = Qk diag(exp(-i*t*d)) Qk^T,  Qk = kron(U, I2), d = repeat(lam, 2)
so per batch element the work collapses to (batched) dense matmuls plus an
elementwise phase rotation; the only batch-dependent transcendental is the
[B, 128] phase table, which is a host-side O(B*128) precompute (the batch-
independent 128x128 expm/eigh constant folding also happens on host).  The
device does all O(B*128^2) work: the complex 128x128 matvecs for the whole
batch, sharded batch-parallel over 8 NeuronCores.

For the ladder operator, M is exactly diagonal (U = I), so the kernel is
    out = P (.) (K @ s)          P[b,k] = exp(-i*dt_b*d_k)   (elementwise)
i.e. per core: 4 fp32 [128x128x128] matmuls + 6 elementwise ops on [128,128].
The phase table replicates jax's complex64 Pade expm scalar recurrence so the
output matches the reference's own rounding to ~1e-6 instead of ~1e-3.
"""

import numpy as np

N = 64
D = 2 * N           # 128: full state dimension (partition dim everywhere)
B = 1024
N_CORES = 8
BS = B // N_CORES   # 128 batch elements per core

_cache = {}


# ---------------------------------------------------------------------------
# host-side constant folding
# ---------------------------------------------------------------------------

def _kick_matrix(a_re, zeta):
    """expm(-i*zeta*kron(a+a^T, sigma_x)) via float64 eigh (real symmetric)."""
    a = a_re.astype(np.float64)
    sx = np.array([[0.0, 1.0], [1.0, 0.0]])
    Hk = np.kron(a + a.T, sx)
    w, V = np.linalg.eigh(Hk)
    return (V * np.exp(-1j * float(zeta) * w)) @ V.T  # complex128 [D, D]


def _phases_pade_c64(t_arr, d_vec):
    """Replicate jax.scipy.linalg.expm (complex64 path) applied to the
    diagonal matrix -1j*t*diag(d_vec): per-element Pade + squaring in
    complex64, so the phases match the reference's rounding (~1e-6)."""
    t_arr = t_arr.astype(np.float32)
    d_vec = d_vec.astype(np.float32)
    theta = (t_arr[:, None] * d_vec[None, :]).astype(np.float32)  # [B, D]
    with np.errstate(divide="ignore"):
        A_L1 = np.max(np.abs(theta), axis=1)                      # [B]
        maxnorm = np.float32(3.925724783138660)
        s = np.maximum(np.float32(0),
                       np.floor(np.log2(A_L1 / maxnorm))).astype(np.float32)
    scale = (np.float32(2.0) ** s).astype(np.float32)
    x = (-1j * (theta / scale[:, None])).astype(np.complex64)

    one = np.complex64(1.0)
    A2 = (x * x).astype(np.complex64)
    A4 = (A2 * A2).astype(np.complex64)
    A6 = (A4 * A2).astype(np.complex64)

    def pade3():
        b = [np.float32(v) for v in (120., 60., 12., 1.)]
        U = (x * (b[3] * A2 + b[1] * one)).astype(np.complex64)
        V = (b[2] * A2 + b[0] * one).astype(np.complex64)
        return U, V

    def pade5():
        b = [np.float32(v) for v in (30240., 15120., 3360., 420., 30., 1.)]
        U = (x * (b[5] * A4 + b[3] * A2 + b[1] * one)).astype(np.complex64)
        V = (b[4] * A4 + b[2] * A2 + b[0] * one).astype(np.complex64)
        return U, V

    def pade7():
        b = [np.float32(v) for v in
             (17297280., 8648640., 1995840., 277200., 25200., 1512., 56., 1.)]
        U = (x * (b[7] * A6 + b[5] * A4 + b[3] * A2 + b[1] * one)).astype(np.complex64)
        V = (b[6] * A6 + b[4] * A4 + b[2] * A2 + b[0] * one).astype(np.complex64)
        return U, V

    conds = np.array([4.258730016922831e-01, 1.880152677804762e+00], np.float32)
    idx = np.digitize(A_L1, conds)                                # [B] in {0,1,2}
    U3, V3 = pade3(); U5, V5 = pade5(); U7, V7 = pade7()
    Uu = np.where(idx[:, None] == 0, U3, np.where(idx[:, None] == 1, U5, U7))
    Vv = np.where(idx[:, None] == 0, V3, np.where(idx[:, None] == 1, V5, V7))
    R = ((Uu + Vv) / (-Uu + Vv)).astype(np.complex64)
    for i in range(int(s.max()) if s.size else 0):
        R = np.where((np.float32(i) < s)[:, None], (R * R).astype(np.complex64), R)
    return R  # complex64 [B, D]


# ---------------------------------------------------------------------------
# device kernel
# ---------------------------------------------------------------------------

def _build(with_rotation):
    """Per-core SPMD kernel.  DRAM I/O (all fp32, [partition, free]):
      st  [D, 2*BS] : [ S_re^T | S_im^T ]   state shard, dim-major
      kw  [D, 3*D]  : [ Wre^T | -Wim^T | Wim^T ]  (lhsT layouts)
      ph  [D, 2*BS] : [ P_re^T | P_im^T ]   phase shard, dim-major
      qt  [D, D]    : Q^T (only if with_rotation)
      out [D, 2*BS] : [ O_re^T | O_im^T ]
    Computes V = W @ S (complex), O = P (.) V, then optionally O = Q @ O.
    """
    import concourse.bass as bass  # noqa: F401
    import concourse.bacc as bacc
    import concourse.tile as tile
    from concourse import mybir

    f32 = mybir.dt.float32
    nc = bacc.Bacc("TRN2", target_bir_lowering=False, debug=False,
                   num_devices=N_CORES)
    st = nc.dram_tensor("st", [D, 2 * BS], f32, kind="ExternalInput").ap()
    kw = nc.dram_tensor("kw", [D, 3 * D], f32, kind="ExternalInput").ap()
    ph = nc.dram_tensor("ph", [D, 2 * BS], f32, kind="ExternalInput").ap()
    if with_rotation:
        qt = nc.dram_tensor("qt", [D, D], f32, kind="ExternalInput").ap()
    out = nc.dram_tensor("out", [D, 2 * BS], f32, kind="ExternalOutput").ap()

    with tile.TileContext(nc) as tc:
        with tc.tile_pool(name="io", bufs=1) as io, \
             tc.tile_pool(name="ps", bufs=1, space="PSUM") as ps, \
             tc.tile_pool(name="tmp", bufs=1) as tmp:
            st_t = io.tile([D, 2 * BS], f32)
            nc.sync.dma_start(st_t[:], st[:])
            ph_t = io.tile([D, 2 * BS], f32)
            nc.sync.dma_start(ph_t[:], ph[:])
            kw_t = io.tile([D, 3 * D], f32)
            nc.scalar.dma_start(kw_t[:], kw[:])
            if with_rotation:
                qt_t = io.tile([D, D], f32)
                nc.scalar.dma_start(qt_t[:], qt[:])

            s_re, s_im = st_t[:, 0:BS], st_t[:, BS:2 * BS]
            p_re, p_im = ph_t[:, 0:BS], ph_t[:, BS:2 * BS]
            w_re, nw_im, w_im = kw_t[:, 0:D], kw_t[:, D:2 * D], kw_t[:, 2 * D:3 * D]

            # V = W @ S  (complex), dim-major: V[k, b]
            v_re = ps.tile([D, BS], f32)
            v_im = ps.tile([D, BS], f32)
            nc.tensor.matmul(v_re[:], w_re, s_re, start=True, stop=False)
            nc.tensor.matmul(v_im[:], w_re, s_im, start=True, stop=False)
            nc.tensor.matmul(v_re[:], nw_im, s_im, start=False, stop=True)
            nc.tensor.matmul(v_im[:], w_im, s_re, start=False, stop=True)

            # O = P (.) V  (complex elementwise)
            o_t = tmp.tile([D, 2 * BS], f32)
            o_re, o_im = o_t[:, 0:BS], o_t[:, BS:2 * BS]
            t1 = tmp.tile([D, BS], f32)
            t2 = tmp.tile([D, BS], f32)
            nc.vector.tensor_mul(t1[:], v_re[:], p_re)
            nc.vector.tensor_mul(t2[:], v_im[:], p_im)
            nc.vector.tensor_sub(o_re, t1[:], t2[:])
            t3 = tmp.tile([D, BS], f32)
            t4 = tmp.tile([D, BS], f32)
            nc.vector.tensor_mul(t3[:], v_im[:], p_re)
            nc.vector.tensor_mul(t4[:], v_re[:], p_im)
            nc.vector.tensor_add(o_im, t3[:], t4[:])

            if with_rotation:
                r_re = ps.tile([D, BS], f32)
                r_im = ps.tile([D, BS], f32)
                nc.tensor.matmul(r_re[:], qt_t[:], o_re, start=True, stop=True)
                nc.tensor.matmul(r_im[:], qt_t[:], o_im, start=True, stop=True)
                f_t = tmp.tile([D, 2 * BS], f32)
                nc.vector.tensor_copy(f_t[:, 0:BS], r_re[:])
                nc.scalar.copy(f_t[:, BS:2 * BS], r_im[:])
                nc.sync.dma_start(out[:], f_t[:])
            else:
                nc.sync.dma_start(out[:], o_t[:])

    nc.compile()
    return nc


def _get_nc(with_rotation):
    key = ("nc", with_rotation)
    if key not in _cache:
        _cache[key] = _build(with_rotation)
    return _cache[key]


# ---------------------------------------------------------------------------
# entry point
# ---------------------------------------------------------------------------

def run(inputs, trace=False):
    from concourse.bass_utils import run_bass_kernel_spmd

    X = np.asarray(inputs["X"], dtype=np.float32)
    s_re = np.asarray(inputs["state_re"], dtype=np.float32)[:, :, 0]  # [B, D]
    s_im = np.asarray(inputs["state_im"], dtype=np.float32)[:, :, 0]
    a_re = np.asarray(inputs["a_re"], dtype=np.float32)
    zeta = float(np.asarray(inputs["zeta"]))
    time = float(np.asarray(inputs["time"]))
    assert X.shape == (B,) and s_re.shape == (B, D) and a_re.shape == (N, N)

    K = _kick_matrix(a_re, zeta)                       # complex128 [D, D]
    M = (a_re @ a_re.T).astype(np.float32)
    diag_M = np.abs(M - np.diag(np.diag(M))).max() == 0.0
    dt = (X + np.float32(time)).astype(np.float32)

    if diag_M:
        # H_rot already diagonal: phases replicate the reference's complex64
        # Pade expm exactly; no eigenbasis rotation needed.
        with_rotation = False
        d_vec = np.repeat(np.diag(M), 2)               # [D]
        P = _phases_pade_c64(dt, d_vec)                # complex64 [B, D]
        W = K
    else:
        # General fallback: eigendecompose M (exact phases; the reference's
        # own complex64 expm error dominates the comparison here).
        with_rotation = True
        lam, U = np.linalg.eigh(M.astype(np.float64))
        d_vec = np.repeat(lam, 2)
        theta = dt.astype(np.float64)[:, None] * d_vec[None, :]
        P = np.exp(-1j * theta).astype(np.complex64)
        Q = np.kron(U, np.eye(2))
        W = Q.T @ K

    W_re = np.ascontiguousarray(W.real.T, dtype=np.float32)   # lhsT [j, k]
    nW_im = np.ascontiguousarray((-W.imag).T, dtype=np.float32)
    W_im = np.ascontiguousarray(W.imag.T, dtype=np.float32)
    kw_np = np.ascontiguousarray(np.concatenate([W_re, nW_im, W_im], axis=1))
    P_re = P.real.astype(np.float32)                   # [B, D]
    P_im = P.imag.astype(np.float32)

    in_maps = []
    for c in range(N_CORES):
        sl = slice(c * BS, (c + 1) * BS)
        st_np = np.ascontiguousarray(
            np.concatenate([s_re[sl].T, s_im[sl].T], axis=1), dtype=np.float32)
        ph_np = np.ascontiguousarray(
            np.concatenate([P_re[sl].T, P_im[sl].T], axis=1), dtype=np.float32)
        m = {"st": st_np, "kw": kw_np, "ph": ph_np}
        if with_rotation:
            m["qt"] = np.ascontiguousarray(Q.T, dtype=np.float32)
        in_maps.append(m)

    nc = _get_nc(with_rotation)
    res = run_bass_kernel_spmd(nc, in_maps, list(range(N_CORES)), trace=trace)

    full = np.empty((B, D), dtype=np.complex64)
    for c in range(N_CORES):
        o = res.results[c]["out"]                      # [D, 2*BS]
        full[c * BS:(c + 1) * BS] = o[:, 0:BS].T + 1j * o[:, BS:2 * BS].T
    return full[:, :, None].astype(np.complex64), res


def kernel(**inputs):
    out, _ = run(inputs)
    return out


# revision 4
# speedup vs baseline: 1.0563x; 1.0563x over previous
"""Trainium2 Bass kernel for the batched quantum-gate problem.

Math: the reference computes, per batch element b,
    out_b = expm(-i*dt_b*H_rot) @ expm(-i*zeta*H_kick) @ s_b
with H_kick = kron(a + a^T, sigma_x), H_rot = kron(a @ a^T, I2), dt = X + time.

H_rot = kron(M, I2) with M = a @ a^T symmetric PSD.  With M = U diag(lam) U^T:
    expm(-i*t*H_rot) = Qk diag(exp(-i*t*d)) Qk^T,  Qk = kron(U, I2), d = repeat(lam, 2)
so per batch element the work collapses to (batched) dense matmuls plus an
elementwise phase rotation; the only batch-dependent transcendental is the
[B, 128] phase table, which is a host-side O(B*128) precompute (the batch-
independent 128x128 expm/eigh constant folding also happens on host).  The
device does all O(B*128^2) work: the complex 128x128 matvecs for the whole
batch, sharded batch-parallel over 8 NeuronCores.

For the ladder operator, M is exactly diagonal (U = I), so the kernel is
    out = P (.) (K @ s)          P[b,k] = exp(-i*dt_b*d_k)   (elementwise)
i.e. per core: 4 fp32 [128x128x128] matmuls + 6 elementwise ops on [128,128].
The phase table replicates jax's complex64 Pade expm scalar recurrence so the
output matches the reference's own rounding to ~1e-6 instead of ~1e-3.
"""

import numpy as np

N = 64
D = 2 * N           # 128: full state dimension (partition dim everywhere)
B = 1024
N_CORES = 8
BS = B // N_CORES   # 128 batch elements per core

_cache = {}


# ---------------------------------------------------------------------------
# host-side constant folding
# ---------------------------------------------------------------------------

def _kick_matrix(a_re, zeta):
    """expm(-i*zeta*kron(a+a^T, sigma_x)) via float64 eigh (real symmetric)."""
    a = a_re.astype(np.float64)
    sx = np.array([[0.0, 1.0], [1.0, 0.0]])
    Hk = np.kron(a + a.T, sx)
    w, V = np.linalg.eigh(Hk)
    return (V * np.exp(-1j * float(zeta) * w)) @ V.T  # complex128 [D, D]


def _phases_pade_c64(t_arr, d_vec):
    """Replicate jax.scipy.linalg.expm (complex64 path) applied to the
    diagonal matrix -1j*t*diag(d_vec): per-element Pade + squaring in
    complex64, so the phases match the reference's rounding (~1e-6)."""
    t_arr = t_arr.astype(np.float32)
    d_vec = d_vec.astype(np.float32)
    theta = (t_arr[:, None] * d_vec[None, :]).astype(np.float32)  # [B, D]
    with np.errstate(divide="ignore"):
        A_L1 = np.max(np.abs(theta), axis=1)                      # [B]
        maxnorm = np.float32(3.925724783138660)
        s = np.maximum(np.float32(0),
                       np.floor(np.log2(A_L1 / maxnorm))).astype(np.float32)
    scale = (np.float32(2.0) ** s).astype(np.float32)
    x = (-1j * (theta / scale[:, None])).astype(np.complex64)

    one = np.complex64(1.0)
    A2 = (x * x).astype(np.complex64)
    A4 = (A2 * A2).astype(np.complex64)
    A6 = (A4 * A2).astype(np.complex64)

    def pade3():
        b = [np.float32(v) for v in (120., 60., 12., 1.)]
        U = (x * (b[3] * A2 + b[1] * one)).astype(np.complex64)
        V = (b[2] * A2 + b[0] * one).astype(np.complex64)
        return U, V

    def pade5():
        b = [np.float32(v) for v in (30240., 15120., 3360., 420., 30., 1.)]
        U = (x * (b[5] * A4 + b[3] * A2 + b[1] * one)).astype(np.complex64)
        V = (b[4] * A4 + b[2] * A2 + b[0] * one).astype(np.complex64)
        return U, V

    def pade7():
        b = [np.float32(v) for v in
             (17297280., 8648640., 1995840., 277200., 25200., 1512., 56., 1.)]
        U = (x * (b[7] * A6 + b[5] * A4 + b[3] * A2 + b[1] * one)).astype(np.complex64)
        V = (b[6] * A6 + b[4] * A4 + b[2] * A2 + b[0] * one).astype(np.complex64)
        return U, V

    conds = np.array([4.258730016922831e-01, 1.880152677804762e+00], np.float32)
    idx = np.digitize(A_L1, conds)                                # [B] in {0,1,2}
    U3, V3 = pade3(); U5, V5 = pade5(); U7, V7 = pade7()
    Uu = np.where(idx[:, None] == 0, U3, np.where(idx[:, None] == 1, U5, U7))
    Vv = np.where(idx[:, None] == 0, V3, np.where(idx[:, None] == 1, V5, V7))
    R = ((Uu + Vv) / (-Uu + Vv)).astype(np.complex64)
    for i in range(int(s.max()) if s.size else 0):
        R = np.where((np.float32(i) < s)[:, None], (R * R).astype(np.complex64), R)
    return R  # complex64 [B, D]


# ---------------------------------------------------------------------------
# device kernel
# ---------------------------------------------------------------------------

def _build_raw():
    """Raw-Bass (no Tile) variant of the diagonal fast path: hand-rolled
    semaphores, no partition-id preamble, no Tile entry/exit barriers.
    DRAM I/O per core (fp32, [partition, free]):
      ws  [D, 3*D + 2*BS] : [ Wre^T | -Wim^T | Wim^T | S_re^T | S_im^T ]
      ph  [D, 2*BS]       : [ P_re^T | P_im^T ]
      out [D, 2*BS]       : [ O_re^T | O_im^T ]
    """
    import concourse.bass as bass
    from concourse import mybir

    f32 = mybir.dt.float32
    nc = bass.Bass("TRN2", debug=False, num_devices=N_CORES,
                   enable_partition_id=False)
    ws = nc.dram_tensor("ws", [D, 3 * D + 2 * BS], f32, kind="ExternalInput").ap()
    ph = nc.dram_tensor("ph", [D, 2 * BS], f32, kind="ExternalInput").ap()
    out = nc.dram_tensor("out", [D, 2 * BS], f32, kind="ExternalOutput").ap()

    with (
        nc.sbuf_tensor([D, 3 * D + 2 * BS], f32) as ws_t,
        nc.sbuf_tensor([D, 2 * BS], f32) as ph_t,
        nc.sbuf_tensor([D, 2 * BS], f32) as o_t,
        nc.sbuf_tensor([D, 4 * BS], f32) as tmp_t,
        nc.psum_tensor([D, BS], f32) as v_re,
        nc.psum_tensor([D, BS], f32) as v_im,
        nc.semaphore("dA") as dA,
        nc.semaphore("dB") as dB,
        nc.semaphore("dO") as dO,
        nc.semaphore("pe") as pe,
        nc.semaphore("dv") as dv,
        nc.Block() as block,
    ):
        w_re = ws_t[:, 0:D]
        nw_im = ws_t[:, D:2 * D]
        w_im = ws_t[:, 2 * D:3 * D]
        s_re = ws_t[:, 3 * D:3 * D + BS]
        s_im = ws_t[:, 3 * D + BS:3 * D + 2 * BS]
        p_re = ph_t[:, 0:BS]
        p_im = ph_t[:, BS:2 * BS]
        o_re = o_t[:, 0:BS]
        o_im = o_t[:, BS:2 * BS]
        t1 = tmp_t[:, 0:BS]
        t2 = tmp_t[:, BS:2 * BS]
        t3 = tmp_t[:, 2 * BS:3 * BS]
        t4 = tmp_t[:, 3 * BS:4 * BS]

        @block.sync
        def _(sync):
            sync.dma_start(out=ws_t[:], in_=ws[:]).then_inc(dA, 16)
            sync.wait_ge(dv, 1)
            sync.dma_start(out=out[:], in_=o_t[:]).then_inc(dO, 16)
            sync.wait_ge(dO, 16)

        @block.scalar
        def _(scalar):
            scalar.dma_start(out=ph_t[:], in_=ph[:]).then_inc(dB, 16)

        @block.tensor
        def _(tensor):
            tensor.wait_ge(dA, 16)
            # interleave so w_re stays loaded for two matmuls; v_re done 3rd
            nc.tensor.matmul(v_re[:], w_re, s_re, start=True, stop=False)
            nc.tensor.matmul(v_im[:], w_re, s_im, start=True, stop=False)
            nc.tensor.matmul(v_re[:], nw_im, s_im, start=False, stop=True
                             ).then_inc(pe, 1)
            nc.tensor.matmul(v_im[:], w_im, s_re, start=False, stop=True
                             ).then_inc(pe, 1)

        @block.vector
        def _(vector):
            vector.wait_ge(dB, 16)
            vector.wait_ge(pe, 1)
            nc.vector.tensor_mul(t1, v_re[:], p_re)
            nc.vector.tensor_mul(t4, v_re[:], p_im)
            vector.wait_ge(pe, 2)
            nc.vector.tensor_mul(t2, v_im[:], p_im)
            nc.vector.tensor_sub(o_re, t1, t2)
            nc.vector.tensor_mul(t3, v_im[:], p_re)
            nc.vector.tensor_add(o_im, t3, t4).then_inc(dv, 1)

    return nc


def _build(with_rotation):
    """Per-core SPMD kernel.  DRAM I/O (all fp32, [partition, free]):
      st  [D, 2*BS] : [ S_re^T | S_im^T ]   state shard, dim-major
      kw  [D, 3*D]  : [ Wre^T | -Wim^T | Wim^T ]  (lhsT layouts)
      ph  [D, 2*BS] : [ P_re^T | P_im^T ]   phase shard, dim-major
      qt  [D, D]    : Q^T (only if with_rotation)
      out [D, 2*BS] : [ O_re^T | O_im^T ]
    Computes V = W @ S (complex), O = P (.) V, then optionally O = Q @ O.
    """
    import concourse.bass as bass  # noqa: F401
    import concourse.bacc as bacc
    import concourse.tile as tile
    from concourse import mybir

    f32 = mybir.dt.float32
    nc = bacc.Bacc("TRN2", target_bir_lowering=False, debug=False,
                   num_devices=N_CORES)
    st = nc.dram_tensor("st", [D, 2 * BS], f32, kind="ExternalInput").ap()
    kw = nc.dram_tensor("kw", [D, 3 * D], f32, kind="ExternalInput").ap()
    ph = nc.dram_tensor("ph", [D, 2 * BS], f32, kind="ExternalInput").ap()
    if with_rotation:
        qt = nc.dram_tensor("qt", [D, D], f32, kind="ExternalInput").ap()
    out = nc.dram_tensor("out", [D, 2 * BS], f32, kind="ExternalOutput").ap()

    with tile.TileContext(nc) as tc:
        with tc.tile_pool(name="io", bufs=1) as io, \
             tc.tile_pool(name="ps", bufs=1, space="PSUM") as ps, \
             tc.tile_pool(name="tmp", bufs=1) as tmp:
            st_t = io.tile([D, 2 * BS], f32)
            nc.sync.dma_start(st_t[:], st[:])
            ph_t = io.tile([D, 2 * BS], f32)
            nc.sync.dma_start(ph_t[:], ph[:])
            kw_t = io.tile([D, 3 * D], f32)
            nc.scalar.dma_start(kw_t[:], kw[:])
            if with_rotation:
                qt_t = io.tile([D, D], f32)
                nc.scalar.dma_start(qt_t[:], qt[:])

            s_re, s_im = st_t[:, 0:BS], st_t[:, BS:2 * BS]
            p_re, p_im = ph_t[:, 0:BS], ph_t[:, BS:2 * BS]
            w_re, nw_im, w_im = kw_t[:, 0:D], kw_t[:, D:2 * D], kw_t[:, 2 * D:3 * D]

            # V = W @ S  (complex), dim-major: V[k, b]
            v_re = ps.tile([D, BS], f32)
            v_im = ps.tile([D, BS], f32)
            nc.tensor.matmul(v_re[:], w_re, s_re, start=True, stop=False)
            nc.tensor.matmul(v_im[:], w_re, s_im, start=True, stop=False)
            nc.tensor.matmul(v_re[:], nw_im, s_im, start=False, stop=True)
            nc.tensor.matmul(v_im[:], w_im, s_re, start=False, stop=True)

            # O = P (.) V  (complex elementwise)
            o_t = tmp.tile([D, 2 * BS], f32)
            o_re, o_im = o_t[:, 0:BS], o_t[:, BS:2 * BS]
            t1 = tmp.tile([D, BS], f32)
            t2 = tmp.tile([D, BS], f32)
            nc.vector.tensor_mul(t1[:], v_re[:], p_re)
            nc.vector.tensor_mul(t2[:], v_im[:], p_im)
            nc.vector.tensor_sub(o_re, t1[:], t2[:])
            t3 = tmp.tile([D, BS], f32)
            t4 = tmp.tile([D, BS], f32)
            nc.vector.tensor_mul(t3[:], v_im[:], p_re)
            nc.vector.tensor_mul(t4[:], v_re[:], p_im)
            nc.vector.tensor_add(o_im, t3[:], t4[:])

            if with_rotation:
                r_re = ps.tile([D, BS], f32)
                r_im = ps.tile([D, BS], f32)
                nc.tensor.matmul(r_re[:], qt_t[:], o_re, start=True, stop=True)
                nc.tensor.matmul(r_im[:], qt_t[:], o_im, start=True, stop=True)
                f_t = tmp.tile([D, 2 * BS], f32)
                nc.vector.tensor_copy(f_t[:, 0:BS], r_re[:])
                nc.scalar.copy(f_t[:, BS:2 * BS], r_im[:])
                nc.sync.dma_start(out[:], f_t[:])
            else:
                nc.sync.dma_start(out[:], o_t[:])

    nc.compile()
    return nc


def _get_nc(with_rotation):
    key = ("nc", with_rotation)
    if key not in _cache:
        _cache[key] = _build(with_rotation) if with_rotation else _build_raw()
    return _cache[key]


# ---------------------------------------------------------------------------
# entry point
# ---------------------------------------------------------------------------

def run(inputs, trace=False):
    from concourse.bass_utils import run_bass_kernel_spmd

    X = np.asarray(inputs["X"], dtype=np.float32)
    s_re = np.asarray(inputs["state_re"], dtype=np.float32)[:, :, 0]  # [B, D]
    s_im = np.asarray(inputs["state_im"], dtype=np.float32)[:, :, 0]
    a_re = np.asarray(inputs["a_re"], dtype=np.float32)
    zeta = float(np.asarray(inputs["zeta"]))
    time = float(np.asarray(inputs["time"]))
    assert X.shape == (B,) and s_re.shape == (B, D) and a_re.shape == (N, N)

    K = _kick_matrix(a_re, zeta)                       # complex128 [D, D]
    M = (a_re @ a_re.T).astype(np.float32)
    diag_M = np.abs(M - np.diag(np.diag(M))).max() == 0.0
    dt = (X + np.float32(time)).astype(np.float32)

    if diag_M:
        # H_rot already diagonal: phases replicate the reference's complex64
        # Pade expm exactly; no eigenbasis rotation needed.
        with_rotation = False
        d_vec = np.repeat(np.diag(M), 2)               # [D]
        P = _phases_pade_c64(dt, d_vec)                # complex64 [B, D]
        W = K
    else:
        # General fallback: eigendecompose M (exact phases; the reference's
        # own complex64 expm error dominates the comparison here).
        with_rotation = True
        lam, U = np.linalg.eigh(M.astype(np.float64))
        d_vec = np.repeat(lam, 2)
        theta = dt.astype(np.float64)[:, None] * d_vec[None, :]
        P = np.exp(-1j * theta).astype(np.complex64)
        Q = np.kron(U, np.eye(2))
        W = Q.T @ K

    W_re = np.ascontiguousarray(W.real.T, dtype=np.float32)   # lhsT [j, k]
    nW_im = np.ascontiguousarray((-W.imag).T, dtype=np.float32)
    W_im = np.ascontiguousarray(W.imag.T, dtype=np.float32)
    kw_np = np.ascontiguousarray(np.concatenate([W_re, nW_im, W_im], axis=1))
    P_re = P.real.astype(np.float32)                   # [B, D]
    P_im = P.imag.astype(np.float32)

    in_maps = []
    for c in range(N_CORES):
        sl = slice(c * BS, (c + 1) * BS)
        st_np = np.ascontiguousarray(
            np.concatenate([s_re[sl].T, s_im[sl].T], axis=1), dtype=np.float32)
        ph_np = np.ascontiguousarray(
            np.concatenate([P_re[sl].T, P_im[sl].T], axis=1), dtype=np.float32)
        if with_rotation:
            m = {"st": st_np, "kw": kw_np, "ph": ph_np,
                 "qt": np.ascontiguousarray(Q.T, dtype=np.float32)}
        else:
            m = {"ws": np.ascontiguousarray(
                     np.concatenate([kw_np, st_np], axis=1), dtype=np.float32),
                 "ph": ph_np}
        in_maps.append(m)

    nc = _get_nc(with_rotation)
    res = run_bass_kernel_spmd(nc, in_maps, list(range(N_CORES)), trace=trace)

    full = np.empty((B, D), dtype=np.complex64)
    for c in range(N_CORES):
        o = res.results[c]["out"]                      # [D, 2*BS]
        full[c * BS:(c + 1) * BS] = o[:, 0:BS].T + 1j * o[:, BS:2 * BS].T
    return full[:, :, None].astype(np.complex64), res


def kernel(**inputs):
    out, _ = run(inputs)
    return out
